# revision 1
# baseline (speedup 1.0000x reference)
"""DGCNN encoder (4x EdgeConv + global max) as a Bass/Tile kernel on 8 TRN2 cores.

Sharding: data-parallel over batch B=4 with a 2-way query split per cloud.
Core c handles cloud c//2. The host rotates each cloud's points by 1024 for
odd cores, so every core runs the SAME program: layers 1-3 are computed for
all 2048 points (needed for the next layer's kNN), layer 4 and the partial
channel max only for the first 1024 points (= this core's half). The final
(B,1,512) is a host-side max of the two per-core partial maxima per cloud.

Per-layer device algorithm (EdgeConv restructured, channel-transposed):
  dist score q[n,j] = 2<f_n,f_j> - |f_j|^2  (selection-equivalent to the
  reference's top_k ordering; one PE matmul with an augmented contraction row)
  -> exact top-20 per row via 3 rounds of DVE max8/match_replace/max_index
  -> zT = (s*Wn) @ f^T, yT = (s*Wc) @ f^T  (BN scale s folded on host)
  -> MT[o,n] = max_k zT[o,idx[n,k]] via gpsimd indirect_copy column gathers
     (1024 idxs per call, k-major) + DVE max accumulation
  -> outT = max(pre, 0.2*pre), pre = MT - zT + yT  (LeakyReLU(0.2); s>0)
outT is directly the next layer's featT (no transposes anywhere).
"""
import numpy as np

import concourse.bass as bass
import concourse.tile as tile
from concourse import bacc, mybir
from concourse.bass_utils import run_bass_kernel_spmd

F32 = mybir.dt.float32
U16 = mybir.dt.uint16

N = 2048          # points per cloud
NQ4 = 1024        # layer-4 query half
K = 20            # neighbors
P = 128           # partitions
NT = N // P       # 16 row tiles
NT4 = NQ4 // P    # 8 row tiles for layer 4
GCH = 1024        # indirect_copy index-chunk size (HW limit)
NEG = -1.0e30
LAYERS = [
    # (Cin, Cout, n_qtiles)
    (3, 64, NT),
    (64, 64, NT),
    (64, 128, NT),
    (128, 256, NT4),
]


def _topk_rounds(nc, pool, q_ap, nbr16, t):
    """Exact top-20 (as top-24, first 20 valid) of each row of q_ap (P, N).

    q_ap lives in PSUM; match_replace writes SBUF copies. Indices (uint16,
    descending by value) land in nbr16[:, 0:24, t] (k-contiguous layout).
    """
    v8 = pool.tile([P, 24], F32, name="v8", tag="v8")
    qm = pool.tile([P, N], F32, name="qm", tag="qmod")
    cur = q_ap
    for r in range(3):
        vr = v8[:, r * 8 : (r + 1) * 8]
        nc.vector.max(out=vr, in_=cur)
        nc.vector.max_index(
            out=nbr16[:, r * 8 : (r + 1) * 8, t], in_max=vr, in_values=q_ap
        )
        if r < 2:
            nc.vector.match_replace(
                out=qm[:], in_to_replace=vr, in_values=cur, imm_value=NEG
            )
            cur = qm[:]


def build_program():
    nc = bacc.Bacc("TRN2", target_bir_lowering=False, debug=False, num_devices=8)

    featT1 = nc.declare_dram_parameter("featT1", [4, N], F32, isOutput=False)
    augK1 = nc.declare_dram_parameter("augK1", [4, N], F32, isOutput=False)
    wnT = {}
    wcT = {}
    for li, (C, O, _) in enumerate(LAYERS, start=1):
        wnT[li] = nc.declare_dram_parameter(f"wnT{li}", [C, O], F32, isOutput=False)
        wcT[li] = nc.declare_dram_parameter(f"wcT{li}", [C, O], F32, isOutput=False)
    pmax_out = nc.declare_dram_parameter("pmax", [P, 5], F32, isOutput=True)

    with tile.TileContext(nc) as tc:
        _build_tc(tc, featT1, augK1, wnT, wcT, pmax_out)
    return nc


def _build_tc(tc, featT1_d, augK1_d, wnT_d, wcT_d, pmax_d):
    nc = tc.nc
    import contextlib

    with contextlib.ExitStack() as ctx:
        const = ctx.enter_context(tc.tile_pool(name="const", bufs=1))
        feats = ctx.enter_context(tc.tile_pool(name="feats", bufs=3))
        auks = ctx.enter_context(tc.tile_pool(name="auks", bufs=3))
        sc2k = ctx.enter_context(tc.tile_pool(name="sc2k", bufs=1))
        zpool = ctx.enter_context(tc.tile_pool(name="zpool", bufs=2))
        gzp = ctx.enter_context(tc.tile_pool(name="gzp", bufs=3))
        small = ctx.enter_context(tc.tile_pool(name="small", bufs=2))
        qpsum = ctx.enter_context(tc.tile_pool(name="qpsum", bufs=1, space="PSUM"))
        zypsum = ctx.enter_context(tc.tile_pool(name="zypsum", bufs=2, space="PSUM"))
        xxpsum = ctx.enter_context(tc.tile_pool(name="xxpsum", bufs=1, space="PSUM"))

        ones_col = const.tile([P, 1], F32, name="ones_col")
        nc.vector.memset(ones_col[:], 1.0)
        pm = const.tile([P, 5], F32, name="pm")
        nc.vector.memset(pm[:], NEG)

        featT1 = feats.tile([4, N], F32, name="featT1", tag="ft_small")
        nc.sync.dma_start(featT1[:], featT1_d[:])
        augK1 = auks.tile([4, N], F32, name="augK1", tag="ak_small")
        nc.sync.dma_start(augK1[:], augK1_d[:])

        # weights: (C, O); O>128 split into output halves, C>64 split into
        # contraction parts (separate tiles so each starts at base partition 0)
        wn_sb = {}
        wc_sb = {}
        for li, (C, O, _) in enumerate(LAYERS, start=1):
            nh = (O + P - 1) // P
            csplits = [(0, C)] if C <= 64 else [(0, 64), (64, C)]
            wn_sb[li] = []
            wc_sb[li] = []
            for h in range(nh):
                o0, o1 = h * P, min((h + 1) * P, O)
                wn_sb[li].append([])
                wc_sb[li].append([])
                for ci, (c0, c1) in enumerate(csplits):
                    wt = const.tile([c1 - c0, o1 - o0], F32, name=f"wn{li}_{h}_{ci}")
                    nc.sync.dma_start(wt[:], wnT_d[li][c0:c1, o0:o1])
                    wn_sb[li][h].append(wt)
                    wt2 = const.tile([c1 - c0, o1 - o0], F32, name=f"wc{li}_{h}_{ci}")
                    nc.sync.dma_start(wt2[:], wcT_d[li][c0:c1, o0:o1])
                    wc_sb[li][h].append(wt2)

        # feat_state: zy = list of lhsT contraction parts (each base partition 0)
        #             dist = list of (lhsT_ap, rhs_ap) contraction parts
        feat_state = {
            "zy": [featT1[0:3, :]],
            "dist": [(featT1[0:4, :], augK1[0:4, :])],
        }

        for li, (C, O, NQT) in enumerate(LAYERS, start=1):
            last = li == 4
            NQ = NQT * P
            nh = (O + P - 1) // P  # output-channel halves (2 for L4)

            # ------- zT / yT matmuls: (O, N) channel-major -------
            zT = []
            yT = []
            for h in range(nh):
                o0, o1 = h * P, min((h + 1) * P, O)
                oc = o1 - o0
                zt = zpool.tile([P, N], F32, name=f"zT{li}_{h}", tag=f"z{h}")
                yt = zpool.tile([P, NQ], F32, name=f"yT{li}_{h}", tag=f"y{h}")
                if oc < P:
                    nc.vector.memset(zt[:], 0.0)
                zT.append(zt)
                yT.append(yt)
                for t in range(NT):
                    zp = zypsum.tile([oc, P], F32, name=f"zp{li}_{h}_{t}", tag="zy")
                    for pi, lhs in enumerate(feat_state["zy"]):
                        nc.tensor.matmul(
                            zp[:],
                            wn_sb[li][h][pi][:],
                            lhs[:, t * P : (t + 1) * P],
                            start=(pi == 0),
                            stop=(pi == len(feat_state["zy"]) - 1),
                        )
                    nc.scalar.copy(zt[0:oc, t * P : (t + 1) * P], zp[:])
                for t in range(NQT):
                    yp = zypsum.tile([oc, P], F32, name=f"yp{li}_{h}_{t}", tag="zy")
                    for pi, lhs in enumerate(feat_state["zy"]):
                        nc.tensor.matmul(
                            yp[:],
                            wc_sb[li][h][pi][:],
                            lhs[:, t * P : (t + 1) * P],
                            start=(pi == 0),
                            stop=(pi == len(feat_state["zy"]) - 1),
                        )
                    nc.scalar.copy(yt[0:oc, t * P : (t + 1) * P], yp[:])
                # c = y - z on own queries (in place into yT)
                nc.vector.tensor_sub(yt[0:oc, :], yt[0:oc, :], zt[0:oc, 0:NQ])

            # ------- dist + topk -------
            nbr16 = small.tile([P, 24, NQT], U16, name=f"nbr{li}", tag="nbr")
            for t in range(NQT):
                q = qpsum.tile([P, N], F32, name=f"q{li}_{t}", tag="q")
                for ch in range(4):
                    cs = bass.ts(ch, 512)
                    nparts = len(feat_state["dist"])
                    for pi, (lhsT, rhs) in enumerate(feat_state["dist"]):
                        nc.tensor.matmul(
                            q[:, cs],
                            lhsT[:, t * P : (t + 1) * P],
                            rhs[:, cs],
                            start=(pi == 0),
                            stop=(pi == nparts - 1),
                        )
                _topk_rounds(nc, small, q[:], nbr16, t)

            # ------- gather index shuffle to flat k-major (i = k*NQ + n) -------
            # entry (p,t,k): flat i = k*NQ + t*128 + p -> row p%16 (+16*rep),
            # col k*(NQT*8) + t*8 + p//16
            gidx = small.tile([P, K * NQ // 16], U16, name=f"gidx{li}", tag="gidx")
            for g in range(8):
                src = nbr16[g * 16 : (g + 1) * 16, 0:K, :].rearrange("p k t -> p (k t)")
                dst = gidx[0:16, :].rearrange("p (kt g2) -> p kt g2", g2=8)[:, :, g]
                nc.sync.dma_start(dst, src)
            for rep in range(1, 8):
                nc.sync.dma_start(gidx[rep * 16 : (rep + 1) * 16, :], gidx[0:16, :])

            # ------- gather + max over k (per 1024-point chunk, per k) -------
            MT = []
            for h in range(nh):
                for half in range(NQ // GCH):
                    m = zpool.tile(
                        [P, GCH], F32, name=f"MT{li}_{h}_{half}", tag=f"M{h}_{half}"
                    )
                    for k in range(K):
                        i0 = k * NQ + half * GCH
                        gz = gzp.tile([P, GCH], F32, name=f"gz{li}", tag="gz")
                        nc.gpsimd.indirect_copy(
                            out=gz[:],
                            data=zT[h][:],
                            idxs=gidx[:, i0 // 16 : (i0 + GCH) // 16],
                            i_know_ap_gather_is_preferred=True,
                        )
                        if k == 0:
                            nc.vector.tensor_copy(out=m[:], in_=gz[:])
                        else:
                            nc.vector.tensor_max(m[:], m[:], gz[:])
                    MT.append((h, half, m))

            # ------- combine: out = lrelu(M - z + y) -------
            if not last:
                C2 = O
                if C2 <= 64:
                    ft = feats.tile([C2 + 1, N], F32, name=f"featT{li+1}a",
                                    tag="ft_small")
                    ft_parts = [(ft, 0, C2)]
                else:
                    fta = feats.tile([64, N], F32, name=f"featT{li+1}a",
                                     tag="ft_small")
                    ftb = feats.tile([C2 - 64 + 1, N], F32, name=f"featT{li+1}b",
                                     tag="ft_b", bufs=1)
                    ft = fta
                    ft_parts = [(fta, 0, 64), (ftb, 64, C2)]

            for h, half, m in MT:
                o0, o1 = h * P, min((h + 1) * P, O)
                oc = o1 - o0
                cslice = slice(half * GCH, (half + 1) * GCH)
                nc.vector.tensor_add(m[0:oc, :], m[0:oc, :], yT[h][0:oc, cslice])
                sc = gzp.tile([P, GCH], F32, name=f"sc{li}", tag="gz")
                nc.scalar.mul(sc[0:oc, :], m[0:oc, :], 0.2)
                nc.vector.tensor_max(m[0:oc, :], m[0:oc, :], sc[0:oc, :])
                # partial channel max over own points (first 1024 columns)
                if half == 0:
                    col = {1: 0, 2: 1, 3: 2}.get(li, 3 + h)
                    nc.vector.tensor_reduce(
                        out=pm[0:oc, col : col + 1],
                        in_=m[0:oc, :],
                        axis=mybir.AxisListType.X,
                        op=mybir.AluOpType.max,
                    )
                if not last:
                    for buf, r0, r1 in ft_parts:
                        rr0 = max(r0, o0)
                        rr1 = min(r1, o1)
                        if rr0 >= rr1:
                            continue
                        if rr0 - r0 == rr0 - o0:
                            nc.vector.tensor_copy(
                                out=buf[rr0 - r0 : rr1 - r0, cslice],
                                in_=m[rr0 - o0 : rr1 - o0, :],
                            )
                        else:
                            # partition-base shift (e.g. rows 64:128 -> 0:64)
                            nc.sync.dma_start(
                                buf[rr0 - r0 : rr1 - r0, cslice],
                                m[rr0 - o0 : rr1 - o0, :],
                            )

            if last:
                break

            # ------- next-layer augK + xxrow -------
            C2 = O
            sq = sc2k.tile([C2, N], F32, name=f"sq{li}", tag="sc2k")
            for buf, r0, r1 in ft_parts:
                nc.scalar.square(sq[r0:r1, :], buf[0 : r1 - r0, :])
            if C2 <= 64:
                ak = auks.tile([C2 + 1, N], F32, name=f"augK{li+1}a", tag="ak_small")
            else:
                aka = auks.tile([64, N], F32, name=f"augK{li+1}a", tag="ak_small")
                akb = auks.tile([C2 - 64 + 1, N], F32, name=f"augK{li+1}b",
                                tag="ak_b", bufs=1)
            for ch in range(4):
                cs = bass.ts(ch, 512)
                xp = xxpsum.tile([1, 512], F32, name=f"xx{li}_{ch}", tag="xx")
                nc.tensor.matmul(xp[:], ones_col[0:C2, :], sq[:, cs])
                if C2 <= 64:
                    nc.scalar.copy(ak[C2 : C2 + 1, cs], xp[:])
                else:
                    nc.scalar.copy(akb[C2 - 64 : C2 - 64 + 1, cs], xp[:])
            if C2 <= 64:
                nc.scalar.mul(ak[0:C2, :], ft[0:C2, :], 2.0)
                nc.vector.memset(ft[C2 : C2 + 1, :], -1.0)
                feat_state = {
                    "zy": [ft[0:C2, :]],
                    "dist": [(ft[0 : C2 + 1, :], ak[0 : C2 + 1, :])],
                }
            else:
                nc.scalar.mul(aka[:], fta[0:64, :], 2.0)
                nc.scalar.mul(akb[0 : C2 - 64, :], ftb[0 : C2 - 64, :], 2.0)
                nc.vector.memset(ftb[C2 - 64 : C2 - 64 + 1, :], -1.0)
                feat_state = {
                    "zy": [fta[0:64, :], ftb[0 : C2 - 64, :]],
                    "dist": [(fta[0:64, :], aka[:]), (ftb[:], akb[:])],
                }

        nc.sync.dma_start(pmax_d[:], pm[:])


def _crow(pi, parts):
    r0 = sum(p.shape[0] for p in parts[:pi])
    return slice(r0, r0 + parts[pi].shape[0])


_NC_CACHE = None
TRACE = False          # set True (e.g. from test.py) to profile the HW run
RUN_KWARGS = {}        # extra kwargs for run_bass_kernel_spmd when tracing
LAST_RESULTS = None    # BassKernelResults of the most recent run


def _get_program():
    global _NC_CACHE
    if _NC_CACHE is None:
        nc = build_program()
        nc.finalize()   # bacc passes: library loads, act tables, ISA codegen
        _NC_CACHE = nc
    return _NC_CACHE


def kernel(**inputs) -> np.ndarray:
    x = np.asarray(inputs["x"], dtype=np.float32)       # (4, 2048, 3)
    B = x.shape[0]
    ws = {i: np.asarray(inputs[f"w{i}"], np.float32) for i in (1, 2, 3, 4)}
    EPS = 1e-5
    in_maps = []
    wmats = {}
    for li, (C, O, _) in enumerate(LAYERS, start=1):
        g = np.asarray(inputs[f"g{li}"], np.float64)
        b = np.asarray(inputs[f"b{li}"], np.float64)
        m = np.asarray(inputs[f"m{li}"], np.float64)
        v = np.asarray(inputs[f"v{li}"], np.float64)
        s = (g / np.sqrt(v + EPS)).astype(np.float32)
        t = (b - m * s).astype(np.float32)
        assert np.all(s > 0) and np.allclose(t, 0.0), "kernel assumes BN shift==0, scale>0"
        w = ws[li] * s[:, None]                           # fold BN scale
        wmats[li] = (
            np.ascontiguousarray(w[:, :C].T),             # wnT (C, O)
            np.ascontiguousarray(w[:, C:].T),             # wcT (C, O)
        )

    for core in range(8):
        b = core // 2
        roll = (core % 2) * NQ4
        xp = np.concatenate([x[b, roll:], x[b, :roll]], axis=0)  # (2048, 3)
        xx = np.sum(xp.astype(np.float32) ** 2, axis=1)
        featT1 = np.concatenate(
            [xp.T, np.full((1, N), -1.0, np.float32)], axis=0
        ).astype(np.float32)
        augK1 = np.concatenate([2.0 * xp.T, xx[None, :]], axis=0).astype(np.float32)
        im = {"featT1": featT1, "augK1": augK1}
        for li in (1, 2, 3, 4):
            im[f"wnT{li}"] = wmats[li][0]
            im[f"wcT{li}"] = wmats[li][1]
        in_maps.append(im)

    nc = _get_program()
    res = run_bass_kernel_spmd(
        nc, in_maps, core_ids=list(range(8)), trace=TRACE, **RUN_KWARGS
    )
    global LAST_RESULTS
    LAST_RESULTS = res

    # reassemble: pm cols = [x1(64), x2(64), x3(128), x4a(128), x4b(128)]
    out = np.empty((B, 1, 512), np.float32)
    for b in range(B):
        vs = []
        for core in (2 * b, 2 * b + 1):
            pmv = res.results[core]["pmax"]
            vs.append(
                np.concatenate(
                    [pmv[0:64, 0], pmv[0:64, 1], pmv[0:128, 2], pmv[0:128, 3],
                     pmv[0:128, 4]]
                )
            )
        out[b, 0] = np.maximum(vs[0], vs[1])
    return out

